# revision 1
# baseline (speedup 1.0000x reference)
"""FDN reverb kernel for 8x TRN2 NeuronCores.

Computes out = y / max|y| with y[t] = x[t] + sum_n a_n * x[t - d_n],
where a_n = (sum_j Q[j, n]) * g[n]  (the MIX=0.5 factor cancels in the
normalization).

Sharding: time axis split into 8 contiguous shards of 1M samples; each
core's input carries a max-delay halo from the previous shard (zeros for
core 0).  On-core layout is partition-major: partition p holds samples
[p*F, p*F + F) of the shard plus a D-sample halo in front, so every
delayed read is a free-axis offset.

Delay taps run on the tensor engine as diagonal-stationary matmuls
accumulating in PSUM.  For full fp32-level accuracy each operand is
split into a bf16 hi/lo pair (x = xh + xl, a = ah + al) and three exact
bf16 matmuls per tap compute ah*xh + ah*xl + al*xh (the dropped al*xl
term is ~2^-18 relative).  The identity tap is added exactly on the DVE
while evacuating PSUM.  A tiny AllGather(max) across the 8 cores yields
the global normalizer; DVE/ACT scale; DMA out.

Set PRECISE=False for a single-pass float32r version (~2x faster PE
phase, ~1.8e-4 max relative error from the PE's ~11-bit rounding).
"""

import numpy as np
import ml_dtypes

import concourse.bacc as bacc
import concourse.bass as bass
import concourse.mybir as mybir
import concourse.tile as tile
from concourse.bass_utils import run_bass_kernel_spmd

# ---- problem constants (hardcoded; must match the reference) ----
SAMPLE_RATE = 48000
DELAYS_SEC = [0.0297, 0.0371, 0.0411, 0.0437, 0.0533, 0.0617, 0.0731, 0.0797]
DELAYS = [int(d * SAMPLE_RATE) for d in DELAYS_SEC]  # [1425,...,3825]
NTAPS = len(DELAYS)  # 8
T = 8388608
N_CORES = 8
T_CORE = T // N_CORES  # 1048576
P = 128
F = T_CORE // P  # 8192 samples per partition row
D = 3840  # halo (>= max delay 3825), kept 128-aligned
TILE = 512  # matmul moving free dim / PSUM bank size (fp32)
NTILES = F // TILE  # 16

PRECISE = True

_cache = {}


def _build_nc():
    fp32 = mybir.dt.float32
    bf16 = mybir.dt.bfloat16
    f32r = mybir.dt.float32r
    xdt = bf16 if PRECISE else f32r

    nc = bacc.Bacc(
        "TRN2",
        target_bir_lowering=False,
        debug=False,
        enable_asserts=False,
        num_devices=N_CORES,
    )

    # inputs: hi/lo bf16 streams (or one f32r stream), stationary diagonals
    xh_d = nc.dram_tensor("xh", [1, D + T_CORE], xdt, kind="ExternalInput")
    if PRECISE:
        xl_d = nc.dram_tensor("xl", [1, D + T_CORE], xdt, kind="ExternalInput")
    dsets = 2 if PRECISE else 1  # diag value sets: a_hi, a_lo
    diags = nc.dram_tensor("diags", [P, dsets * NTAPS * P], xdt, kind="ExternalInput")
    if PRECISE:
        # full-fp32 a_lo values for the taps whose al*xh pass runs on DVE
        alv = nc.dram_tensor("alv", [P, NTAPS], fp32, kind="ExternalInput")
    ident = nc.dram_tensor("ident", [P, P], fp32, kind="ExternalInput")
    out = nc.dram_tensor("out", [1, T_CORE], fp32, kind="ExternalOutput")

    def shard_ap(t, c0, c1):
        # columns [c0, c1) of the overlapped [128, D+F] row view
        return bass.AP(tensor=t, offset=c0, ap=[[F, P], [1, c1 - c0]])

    # DMA-in column chunks: small leading chunks so the PE can start early
    bounds = [0, 640, 1664, 2944, 4480, 6016, 8000, 10016, 12032]
    XCH = list(zip(bounds[:-1], bounds[1:]))

    with tile.TileContext(nc) as tc:
        with (
            tc.tile_pool(name="xpool", bufs=1) as xpool,
            tc.tile_pool(name="ypool", bufs=1) as ypool,
            tc.tile_pool(name="dpool", bufs=1) as dpool,
            tc.tile_pool(name="spool", bufs=1) as spool,
            tc.tile_pool(name="psum", bufs=7, space="PSUM") as psum_pool,
            tc.tile_pool(name="psumt", bufs=1, space="PSUM") as psumt_pool,
            tc.tile_pool(name="dram", bufs=1, space="DRAM") as dram_pool,
        ):
            xh_t = xpool.tile([P, D + F], xdt)
            xl_t = xpool.tile([P, D + F], xdt, name="xl_t") if PRECISE else None
            y_tile = ypool.tile([P, F], fp32)
            diag_t = dpool.tile([P, dsets * NTAPS * P], xdt)
            alv_t = dpool.tile([P, NTAPS], fp32, name="alv_t") if PRECISE else None
            ident_t = dpool.tile([P, P], fp32, name="ident_t")
            stats = spool.tile([P, NTILES], fp32)
            m_loc = spool.tile([P, 1], fp32)
            m_row = spool.tile([1, P], fp32)
            pt = psumt_pool.tile([1, P], fp32, name="pt")
            inv_b = spool.tile([P, 1], fp32)
            cc_sb = spool.tile([1, 8], fp32)
            g_all = spool.tile([P, 8 * N_CORES], fp32)

            cc_in = dram_pool.tile([1, 8], fp32)
            cc_out = dram_pool.tile([N_CORES, 8], fp32, addr_space="Shared")

            # the first matmul (largest delay tap) needs only its own diag
            # slice and the first x chunk — issue those first
            first_tap = max(range(NTAPS), key=lambda n: DELAYS[n])
            fs0, fs1 = first_tap * P, (first_tap + 1) * P
            nhalf = dsets * NTAPS * P // 2
            nc.sync.dma_start(diag_t[:, fs0:fs1], diags.ap()[:, fs0:fs1])
            for i, (c0, c1) in enumerate(XCH):
                nc.sync.dma_start(xh_t[:, c0:c1], shard_ap(xh_d, c0, c1))
                if PRECISE:
                    nc.sync.dma_start(xl_t[:, c0:c1], shard_ap(xl_d, c0, c1))
                if i == 0:
                    if fs0 > 0:
                        nc.sync.dma_start(diag_t[:, 0:fs0], diags.ap()[:, 0:fs0])
                    if fs1 < nhalf:
                        nc.sync.dma_start(
                            diag_t[:, fs1:nhalf], diags.ap()[:, fs1:nhalf]
                        )
                    if dsets > 1:
                        nc.sync.dma_start(
                            diag_t[:, nhalf:], diags.ap()[:, nhalf:]
                        )
                    if PRECISE:
                        nc.sync.dma_start(alv_t[:], alv.ap())
                    nc.sync.dma_start(ident_t[:], ident.ap())

            # warm up the PE (HAM/pstate ramp) with junk matmuls into the
            # scratch psum bank while the input DMAs stream; pt is fully
            # overwritten later by the start=True transpose
            for _ in range(10):
                nc.tensor.matmul(
                    pt[:], diag_t[:, fs0 : fs0 + 1], diag_t[:, fs0 : fs0 + P],
                    start=True, stop=True,
                )

            # delay taps: diagonal matmuls accumulating in PSUM.
            # Descending delay order so the first matmuls of a tile only
            # need the earliest input columns (x chunks stream in behind).
            # The al*xh pass of the 4 shortest-delay taps runs on the DVE
            # instead (full-fp32 scalars) to rebalance PE (~73us) vs DVE
            # (~66us) busy time.
            order = sorted(range(NTAPS), key=lambda n: -DELAYS[n])

            def tile_plan(j):
                # the last tile keeps everything on the PE so its (serial)
                # DVE evac chain off the critical tail stays short
                dve_c = set(order[-4:]) if (PRECISE and j < NTILES - 1) else set()
                passes = []  # (diag_set, tap, x_tile)
                for n in order:
                    passes.append((0, n, xh_t))  # ah * xh
                    if PRECISE:
                        passes.append((0, n, xl_t))  # ah * xl
                        if n not in dve_c:
                            passes.append((1, n, xh_t))  # al * xh
                return passes, dve_c

            for j in range(NTILES):
                ps = psum_pool.tile([P, TILE], fp32, tag="ps", name=f"ps_{j}")
                base = D + j * TILE
                passes, dve_c_taps = tile_plan(j)
                for i, (s, n, xt) in enumerate(passes):
                    lhsT = diag_t[:, (s * NTAPS + n) * P : (s * NTAPS + n + 1) * P]
                    rhs = xt[:, base - DELAYS[n] : base - DELAYS[n] + TILE]
                    nc.tensor.matmul(
                        ps[:], lhsT, rhs,
                        start=(i == 0), stop=(i == len(passes) - 1),
                    )
                # evacuate PSUM -> SBUF adding the exact identity (x=xh+xl)
                ysl = y_tile[:, j * TILE : (j + 1) * TILE]
                nc.vector.scalar_tensor_tensor(
                    ysl, ps[:], 1.0, xh_t[:, base : base + TILE],
                    op0=mybir.AluOpType.mult, op1=mybir.AluOpType.add,
                )
                if PRECISE:
                    nc.vector.scalar_tensor_tensor(
                        ysl, ysl, 1.0, xl_t[:, base : base + TILE],
                        op0=mybir.AluOpType.mult, op1=mybir.AluOpType.add,
                    )
                    for n in dve_c_taps:
                        sh = slice(base - DELAYS[n], base - DELAYS[n] + TILE)
                        nc.vector.scalar_tensor_tensor(
                            ysl, xh_t[:, sh], alv_t[:, n : n + 1], ysl,
                            op0=mybir.AluOpType.mult, op1=mybir.AluOpType.add,
                        )
                nc.vector.tensor_reduce(
                    stats[:, j : j + 1], ysl,
                    axis=mybir.AxisListType.X, op=mybir.AluOpType.max,
                    apply_absolute_value=True,
                )

            # local max: tiles 0..14 are reduced+gathered+maxed early (all
            # hidden under the PE phase); the last tile's stats column is
            # partition-transposed on the (by then idle) PE and reduced into
            # a second slot of cc_sb -- the post-AG reduce maxes over both.
            nc.vector.memset(cc_sb[:], 0.0)
            nc.vector.tensor_reduce(
                m_loc[:, 0:1], stats[:, 0 : NTILES - 1],
                axis=mybir.AxisListType.X, op=mybir.AluOpType.max,
            )
            nc.sync.dma_start(m_row[0:1, :], m_loc[:, 0:1])
            nc.vector.tensor_reduce(
                cc_sb[0:1, 0:1], m_row[0:1, :],
                axis=mybir.AxisListType.X, op=mybir.AluOpType.max,
            )
            nc.tensor.transpose(pt[:], stats[:, NTILES - 1 : NTILES], ident_t[:])
            nc.vector.tensor_reduce(
                cc_sb[0:1, 1:2], pt[:],
                axis=mybir.AxisListType.X, op=mybir.AluOpType.max,
            )

            # global max across cores: AllGather the 8 local maxima
            nc.sync.dma_start(cc_in[:], cc_sb[:])
            nc.gpsimd.collective_compute(
                "AllGather",
                mybir.AluOpType.bypass,
                replica_groups=[list(range(N_CORES))],
                ins=[cc_in[:].opt()],
                outs=[cc_out[:].opt()],
            )
            # broadcast-read all 64 gathered floats into every partition
            nc.sync.dma_start(
                g_all[:],
                bass.AP(tensor=cc_out.tensor, offset=0, ap=[[0, P], [1, 8 * N_CORES]]),
            )
            nc.vector.tensor_reduce(
                inv_b[:], g_all[:], axis=mybir.AxisListType.X, op=mybir.AluOpType.max
            )
            nc.vector.reciprocal(inv_b[:], inv_b[:])

            # scale + store (DVE-heavy split; small first chunk so the first
            # output DMA starts as soon as possible after the collective)
            SCHUNKS = [
                ("v", 0, 256), ("v", 256, 1280), ("v", 1280, 2304),
                ("v", 2304, 3328), ("v", 3328, 4352), ("v", 4352, 5120),
                ("a", 5120, 6144), ("a", 6144, 7168), ("a", 7168, 8192),
            ]
            for eng, c0, c1 in SCHUNKS:
                ysl = y_tile[:, c0:c1]
                if eng == "v":
                    nc.vector.tensor_scalar_mul(ysl, ysl, inv_b[:, 0:1])
                else:
                    nc.scalar.mul(ysl, ysl, inv_b[:, 0:1])
                nc.sync.dma_start(shard_ap(out, c0, c1), ysl)

    nc.compile()
    return nc


def _prep_inputs(input_sig, feedback_gain, orthogonal_matrix):
    x = np.ascontiguousarray(np.asarray(input_sig, dtype=np.float32)).reshape(T)
    g = np.asarray(feedback_gain, dtype=np.float32)
    q = np.asarray(orthogonal_matrix, dtype=np.float32)
    coeff = (q.sum(axis=0) * g).astype(np.float32)  # [8]

    xpad = np.concatenate([np.zeros(D, np.float32), x])  # [D + T]
    idx = np.arange(P)

    if PRECISE:
        bf = ml_dtypes.bfloat16
        xh = xpad.astype(bf)
        xl = (xpad - xh.astype(np.float32)).astype(bf)
        ah = coeff.astype(bf)
        al_f32 = (coeff - ah.astype(np.float32)).astype(np.float32)
        al = al_f32.astype(bf)
        diags = np.zeros((P, 2 * NTAPS * P), dtype=bf)
        for n in range(NTAPS):
            diags[idx, n * P + idx] = ah[n]
            diags[idx, (NTAPS + n) * P + idx] = al[n]
        alv = np.tile(al_f32.reshape(1, NTAPS), (P, 1)).astype(np.float32)
    else:
        xh = xpad
        ah = coeff
        diags = np.zeros((P, NTAPS * P), dtype=np.float32)
        for n in range(NTAPS):
            diags[idx, n * P + idx] = ah[n]

    ident = np.eye(P, dtype=np.float32)
    in_maps = []
    for c in range(N_CORES):
        sl = slice(c * T_CORE, c * T_CORE + D + T_CORE)
        m = {
            "xh": np.ascontiguousarray(xh[sl]).reshape(1, D + T_CORE),
            "diags": diags,
        }
        m["ident"] = ident
        if PRECISE:
            m["xl"] = np.ascontiguousarray(xl[sl]).reshape(1, D + T_CORE)
            m["alv"] = alv
        in_maps.append(m)
    return in_maps


def _run(in_maps, trace=False):
    if "nc" not in _cache:
        _cache["nc"] = _build_nc()
    nc = _cache["nc"]
    res = run_bass_kernel_spmd(
        nc, in_maps, core_ids=list(range(N_CORES)), trace=trace
    )
    outs = [r["out"].reshape(T_CORE) for r in res.results]
    full = np.concatenate(outs).reshape(1, T)
    return full, res


def kernel(input_sig, feedback_gain, orthogonal_matrix):
    in_maps = _prep_inputs(input_sig, feedback_gain, orthogonal_matrix)
    try:
        full, _ = _run(in_maps, trace=False)
    except Exception:
        # one retry: a freshly-attached terminal occasionally reports a
        # transient device-unrecoverable error on the first execution
        full, _ = _run(in_maps, trace=False)
    return full



# revision 15
# speedup vs baseline: 1.5965x; 1.5965x over previous
"""FDN reverb kernel for 8x TRN2 NeuronCores.

Computes out = y / max|y| with y[t] = x[t] + sum_n a_n * x[t - d_n],
where a_n = (sum_j Q[j, n]) * g[n]  (the MIX=0.5 factor cancels in the
normalization).

Sharding: time axis split into 8 contiguous shards of 1M samples; each
core's input carries a max-delay halo from the previous shard (zeros for
core 0).  On-core layout is partition-major: partition p holds samples
[p*F, p*F + F) of the shard plus a D-sample halo in front, so every
delayed read is a free-axis offset.

Single-pass bf16 data path (rel err ~1e-3, gate is 2e-2): x streams in
as one bf16 tensor (3.1 MB/core with halos).  Per 512-col tile the work
is spread over all four compute engines: most taps + the identity run as
diagonal-stationary bf16 matmuls accumulating in PSUM (PE), the ACT
engine evacuates PSUM -> bf16 y, one small-coefficient tap runs on the
DVE (4x-mode tensor_scalar product + 2x-mode add) and one or two on
GPSIMD (fused scalar_tensor_tensor), and the per-tile abs-max reduction
runs on the DVE.  The last-processed tile stays PE+DVE so the serial
chain into the collective is short.  Each core's [P,1] running max is
DMA-transposed straight to DRAM, one AllGather exchanges [8,128] maxima,
a strided read brings them back as [P,8]; DVE+ACT then scale into an
fp32 staging buffer chunk-by-chunk while the output DMA streams behind.
"""

import numpy as np
import ml_dtypes

import concourse.bacc as bacc
import concourse.bass as bass
import concourse.mybir as mybir
import concourse.tile as tile
from concourse.bass_utils import run_bass_kernel_spmd

# ---- problem constants (hardcoded; must match the reference) ----
SAMPLE_RATE = 48000
DELAYS_SEC = [0.0297, 0.0371, 0.0411, 0.0437, 0.0533, 0.0617, 0.0731, 0.0797]
DELAYS = [int(d * SAMPLE_RATE) for d in DELAYS_SEC]  # [1425,...,3825]
NTAPS = len(DELAYS)  # 8
T = 8388608
N_CORES = 8
T_CORE = T // N_CORES  # 1048576
P = 128
F = T_CORE // P  # 8192 samples per partition row
D = 3840  # halo (>= max delay 3825), kept 128-aligned
TILE = 512  # matmul moving free dim / PSUM bank size (fp32)
NTILES = F // TILE  # 16

# Per-tile engine plan.  evac: 'act' = ACT copies PSUM->y (identity runs
# as a 9th PE diag pass); 'dve' = DVE STT evacuates with the identity
# folded in.  dve/gps: taps offloaded from the PE to the DVE
# (tensor_scalar 4x product + tensor_tensor 2x merge) or to GPSIMD
# (fused STT).  Offloaded taps are the smallest-|coeff| ones so the
# extra bf16 rounding is negligible.  delays: 0:1425 1:1780 2:1972
# 3:2097 4:2558 5:2961 6:3508 7:3825
def _plan(j):
    if j == NTILES - 1:
        return dict(evac="dve", dve=(), ap=())
    if j >= 13:
        # processed last: keep every queue to the bare minimum so the
        # compute phase doesn't end in a serial cross-engine chain
        return dict(evac="act", dve=(), ap=())
    if j >= 6:
        return dict(evac="act", dve=(0,), ap=(1,))
    return dict(evac="act", dve=(0, 6), ap=(1,))


# input column chunks: small leading chunk so the PE can start early
XBOUNDS = [0, 576, 1600, 2624, 3648, 4480, 6016, 8000, 10016, 12032]

# scale+store chunks: (engine, c0, c1); first small so the out DMA
# starts right after the collective
SCHUNKS = [
    ("v", 0, 128),
    ("a", 128, 640),
    ("v", 640, 1664),
    ("a", 1664, 2944),
    ("v", 2944, 4224),
    ("a", 4224, 5504),
    ("v", 5504, 6784),
    ("a", 6784, 8192),
]

N_WARMUP = 40  # junk matmuls to ramp the PE p-state while inputs stream

_cache = {}


def _build_nc():
    fp32 = mybir.dt.float32
    bf16 = mybir.dt.bfloat16

    nc = bacc.Bacc(
        "TRN2",
        target_bir_lowering=False,
        debug=False,
        enable_asserts=False,
        num_devices=N_CORES,
    )

    xh_d = nc.dram_tensor("xh", [1, D + T_CORE], bf16, kind="ExternalInput")
    # diag slot k = coeff of the k-th tap in descending-delay order;
    # slot 8 = identity (1.0)
    diags = nc.dram_tensor("diags", [P, (NTAPS + 1) * P], bf16, kind="ExternalInput")
    alv = nc.dram_tensor("alv", [P, NTAPS], fp32, kind="ExternalInput")
    out = nc.dram_tensor("out", [1, T_CORE], fp32, kind="ExternalOutput")

    def shard_ap(t, c0, c1):
        # columns [c0, c1) of the overlapped [128, D+F] row view
        return bass.AP(tensor=t, offset=c0, ap=[[F, P], [1, c1 - c0]])

    XCH = list(zip(XBOUNDS[:-1], XBOUNDS[1:]))

    with tile.TileContext(nc) as tc:
        with (
            tc.tile_pool(name="xpool", bufs=1) as xpool,
            tc.tile_pool(name="ypool", bufs=1) as ypool,
            tc.tile_pool(name="opool", bufs=1) as opool,
            tc.tile_pool(name="dpool", bufs=1) as dpool,
            tc.tile_pool(name="spool", bufs=1) as spool,
            tc.tile_pool(name="tpool", bufs=34) as tpool,
            tc.tile_pool(name="psum", bufs=7, space="PSUM") as psum_pool,
            tc.tile_pool(name="psumw", bufs=1, space="PSUM") as psumw_pool,
            tc.tile_pool(name="dram", bufs=1, space="DRAM") as dram_pool,
        ):
            xh_t = xpool.tile([P, D + F], bf16)
            y_tile = ypool.tile([P, F], bf16)
            yo_tile = opool.tile([P, F], fp32)
            diag_t = dpool.tile([P, (NTAPS + 1) * P], bf16)
            alv_t = dpool.tile([P, NTAPS], fp32, name="alv_t")
            stats = spool.tile([P, NTILES], fp32)
            m_loc = spool.tile([P, 2], fp32)
            inv_b = spool.tile([P, 1], fp32)
            g_all = spool.tile([P, N_CORES * P], fp32)
            pw = psumw_pool.tile([1, P], fp32, name="pw")

            cc_in = dram_pool.tile([1, P], fp32)
            cc_out = dram_pool.tile([N_CORES, P], fp32, addr_space="Shared")

            # warm up the PE (p-state ramp) with junk matmuls on a
            # memset column (no DMA dependency) while the inputs stream;
            # one accumulation group so the junk runs back-to-back
            wjunk = dpool.tile([P, P], bf16, name="wjunk")
            nc.vector.memset(wjunk[:], 0.0)
            for w in range(N_WARMUP):
                nc.tensor.matmul(
                    pw[:],
                    wjunk[:, 0:1],
                    wjunk[:],
                    start=(w == 0),
                    stop=(w == N_WARMUP - 1),
                )

            # input stream: x chunk 0, then a priority slice of the diag
            # matrix covering the first (largest-delay) taps, then the rest
            # of x; the full diags / coefficient vector follow early x
            nc.sync.dma_start(xh_t[:, XCH[0][0] : XCH[0][1]],
                              shard_ap(xh_d, XCH[0][0], XCH[0][1]))
            nc.sync.dma_start(diag_t[:, 0 : 4 * P], diags.ap()[:, 0 : 4 * P])
            nc.sync.dma_start(alv_t[:], alv.ap())
            for i, (c0, c1) in enumerate(XCH[1:], start=1):
                nc.sync.dma_start(xh_t[:, c0:c1], shard_ap(xh_d, c0, c1))
                if i == 1:
                    nc.sync.dma_start(
                        diag_t[:, 4 * P :], diags.ap()[:, 4 * P :]
                    )

            # Descending delay order so the first matmuls of a tile only
            # need the earliest input columns.  Diag slot k holds the k-th
            # tap in this order so the priority diag DMA covers the taps
            # needed first.
            order = sorted(range(NTAPS), key=lambda n: -DELAYS[n])
            slot_of = {n: k for k, n in enumerate(order)}
            slot_of["id"] = NTAPS

            # tile 15 (PE-only, DVE-evac'd) is processed before tiles
            # 12..14 so the serial chain behind the last PE pass is the
            # short ACT->DVE one; stats columns are indexed by position.
            tile_order = list(range(12)) + [15, 12, 13, 14]

            # Tap products are hoisted ahead of the tile loop: they only
            # depend on the x stream, and the DVE/ACT engines are idle
            # while the first PSUM banks fill.  The "ap" channel products
            # run on ACT; a two-tap bundle is pre-merged with one (also
            # hoisted) DVE add so the Pool engine pays one merge per tile.
            tmps = {}
            apm = {}
            for j in tile_order:
                plan = _plan(j)
                for n in plan["dve"]:
                    sh = slice(D + j * TILE - DELAYS[n], D + j * TILE - DELAYS[n] + TILE)
                    tmp = tpool.tile([P, TILE], bf16, tag="tmp", name=f"tmp_{j}_{n}")
                    nc.vector.tensor_scalar_mul(
                        tmp[:], xh_t[:, sh], alv_t[:, n : n + 1]
                    )
                    tmps[(j, n)] = tmp
                aps = plan["ap"]
                if aps:
                    parts = []
                    for n in aps:
                        sh = slice(D + j * TILE - DELAYS[n], D + j * TILE - DELAYS[n] + TILE)
                        tmp = tpool.tile([P, TILE], bf16, tag="tmp", name=f"ap_{j}_{n}")
                        nc.scalar.mul(tmp[:], xh_t[:, sh], alv_t[:, n : n + 1])
                        parts.append(tmp)
                    if len(parts) == 2:
                        nc.vector.tensor_tensor(
                            parts[0][:], parts[0][:], parts[1][:],
                            op=mybir.AluOpType.add,
                        )
                    apm[j] = parts[0]

            for pos, j in enumerate(tile_order):
                plan = _plan(j)
                dve_taps, ap_taps = plan["dve"], plan["ap"]
                pe_taps = [n for n in order if n not in dve_taps and n not in ap_taps]
                ps = psum_pool.tile([P, TILE], fp32, tag="ps", name=f"ps_{j}")
                base = D + j * TILE
                ysl = y_tile[:, j * TILE : (j + 1) * TILE]
                st = stats[:, pos : pos + 1]

                act_evac = plan["evac"] == "act"
                npass = len(pe_taps) + (1 if act_evac else 0)
                for i, n in enumerate(pe_taps):
                    lhsT = diag_t[:, slot_of[n] * P : (slot_of[n] + 1) * P]
                    rhs = xh_t[:, base - DELAYS[n] : base - DELAYS[n] + TILE]
                    nc.tensor.matmul(
                        ps[:], lhsT, rhs, start=(i == 0), stop=(i == npass - 1)
                    )
                if act_evac:
                    # identity tap on the PE; ACT evacuates the finished bank
                    nc.tensor.matmul(
                        ps[:],
                        diag_t[:, NTAPS * P : (NTAPS + 1) * P],
                        xh_t[:, base : base + TILE],
                        start=False,
                        stop=True,
                    )
                    nc.scalar.copy(ysl, ps[:])
                else:
                    # identity folded into the DVE evac
                    nc.vector.scalar_tensor_tensor(
                        ysl,
                        ps[:],
                        1.0,
                        xh_t[:, base : base + TILE],
                        op0=mybir.AluOpType.mult,
                        op1=mybir.AluOpType.add,
                    )
                if j in apm:
                    nc.gpsimd.tensor_tensor(
                        ysl, ysl, apm[j][:], op=mybir.AluOpType.add
                    )
                for n in dve_taps:
                    nc.vector.tensor_tensor(
                        ysl, ysl, tmps[(j, n)][:], op=mybir.AluOpType.add
                    )
                nc.vector.tensor_reduce(
                    st,
                    ysl,
                    axis=mybir.AxisListType.X,
                    op=mybir.AluOpType.max,
                    apply_absolute_value=True,
                )

            # local max: the first 15 processed tiles are pre-reduced early
            # (hidden under compute); the last stat column is merged with a
            # tiny tensor_tensor max, then the [P,1] column goes straight to
            # DRAM as this core's AllGather contribution.
            nc.vector.tensor_reduce(
                m_loc[:, 0:1],
                stats[:, 0 : NTILES - 1],
                axis=mybir.AxisListType.X,
                op=mybir.AluOpType.max,
            )
            nc.vector.tensor_tensor(
                m_loc[:, 1:2],
                m_loc[:, 0:1],
                stats[:, NTILES - 1 : NTILES],
                op=mybir.AluOpType.max,
            )
            nc.sync.dma_start(cc_in[0:1, :], m_loc[:, 1:2])

            # global max across cores: AllGather the 8 [1,128] local maxima
            nc.gpsimd.collective_compute(
                "AllGather",
                mybir.AluOpType.bypass,
                replica_groups=[list(range(N_CORES))],
                ins=[cc_in[:].opt()],
                outs=[cc_out[:].opt()],
            )
            # broadcast-read the full [8,128] gather into every partition
            nc.sync.dma_start(
                g_all[:],
                bass.AP(
                    tensor=cc_out.tensor, offset=0, ap=[[0, P], [1, N_CORES * P]]
                ),
            )
            nc.vector.tensor_reduce(
                inv_b[:], g_all[:], axis=mybir.AxisListType.X, op=mybir.AluOpType.max
            )
            nc.vector.reciprocal(inv_b[:], inv_b[:])

            # scale + store (DVE/ACT split; small first chunk so the first
            # output DMA starts as soon as possible after the collective)
            for eng, c0, c1 in SCHUNKS:
                ysl = y_tile[:, c0:c1]
                osl = yo_tile[:, c0:c1]
                if eng == "v":
                    nc.vector.tensor_scalar_mul(osl, ysl, inv_b[:, 0:1])
                else:
                    nc.scalar.mul(osl, ysl, inv_b[:, 0:1])
                nc.sync.dma_start(shard_ap(out, c0, c1), osl)

    nc.compile()
    return nc


def _prep_inputs(input_sig, feedback_gain, orthogonal_matrix):
    x = np.ascontiguousarray(np.asarray(input_sig, dtype=np.float32)).reshape(T)
    g = np.asarray(feedback_gain, dtype=np.float32)
    q = np.asarray(orthogonal_matrix, dtype=np.float32)
    coeff = (q.sum(axis=0) * g).astype(np.float32)  # [8]

    bf = ml_dtypes.bfloat16
    xpad = np.concatenate([np.zeros(D, np.float32), x])  # [D + T]
    xh = xpad.astype(bf)
    idx = np.arange(P)

    ab = coeff.astype(bf)
    order = sorted(range(NTAPS), key=lambda n: -DELAYS[n])
    diags = np.zeros((P, (NTAPS + 1) * P), dtype=bf)
    for k, n in enumerate(order):
        diags[idx, k * P + idx] = ab[n]
    diags[idx, NTAPS * P + idx] = bf(1.0)
    alv = np.tile(coeff.reshape(1, NTAPS), (P, 1)).astype(np.float32)

    in_maps = []
    for c in range(N_CORES):
        sl = slice(c * T_CORE, c * T_CORE + D + T_CORE)
        m = {
            "xh": np.ascontiguousarray(xh[sl]).reshape(1, D + T_CORE),
            "diags": diags,
            "alv": alv,
        }
        in_maps.append(m)
    return in_maps


def _run(in_maps, trace=False):
    if "nc" not in _cache:
        _cache["nc"] = _build_nc()
    nc = _cache["nc"]
    res = run_bass_kernel_spmd(
        nc, in_maps, core_ids=list(range(N_CORES)), trace=trace
    )
    outs = [r["out"].reshape(T_CORE) for r in res.results]
    full = np.concatenate(outs).reshape(1, T)
    return full, res


def kernel(input_sig, feedback_gain, orthogonal_matrix):
    in_maps = _prep_inputs(input_sig, feedback_gain, orthogonal_matrix)
    try:
        full, _ = _run(in_maps, trace=False)
    except Exception:
        # one retry: a freshly-attached terminal occasionally reports a
        # transient device-unrecoverable error on the first execution
        full, _ = _run(in_maps, trace=False)
    return full
